# revision 18
# baseline (speedup 1.0000x reference)
"""Trainium2 Bass kernel for nn_Encoder_LSTM (4x LSTMCell with zero state over
packed ragged tokens).

Math (from the reference): all rows independent; for each output row j with
source row s(j) (the ragged gather), and each of 4 layers:
    gates = x @ W_ih^T + (b_ih + b_hh);  i, f, g, o = split(gates)
    c = sigmoid(i) * tanh(g);  h = sigmoid(o) * tanh(c)      (f is unused)
Outputs: (output=h4, h1, c1, h2, c2, h3, c3, h4, c4), each [sum(bs), 512] fp32.

v2 strategy (vs the v1 slab kernel):
  - Compute each of the U=16448 distinct source rows once; core c takes rows
    c::8 (2056 rows = 17 tiles of 128). Store ONLY distinct rows, in bf16;
    the host expands duplicates + upcasts to f32.
  - tanh-only activation path: sigmoid(z) = (tanh(z/2)+1)/2. The i/o gate
    weight+bias columns are pre-scaled by 0.5 on the host so ONE tanh over
    all 1536 packed gates [i,o,g] gives t_i, t_o, t_g. Then
        c_raw = (t_i + 1) * t_g          ( = 2c )
        h_raw = (t_o + 1) * tanh(0.5*c_raw)   ( = 2h )
    each ONE fused DVE scalar_tensor_tensor in bf16 (2x mode). The factor 2
    on h is folded into the next layer's weights (x0.5), and the stored
    h_raw/c_raw are halved on the host.
  - Host pre-transposes x per core, so layer-1 lhsT slices come straight from
    SBUF. Inter-layer transposes are REGULAR matmuls against a bf16 identity
    (fast warm-PE path, f32 PSUM out); ACT copies PSUM -> bf16 SBUF.
  - Bias add (free-dim varying, so not expressible as ACT per-partition
    bias) is one DVE tensor_tensor from PSUM per layer.
  - Stores: one [128, 8*512] bf16 DMA per tile (8KB/partition contiguous).
"""

import sys

if "/opt/trn_rl_repo" not in sys.path:
    sys.path.insert(0, "/opt/trn_rl_repo")

import numpy as np
import ml_dtypes

P = 128
H = 512
G = 1536          # 3 packed gates [i, o, g] * 512
J = 8             # fused outputs [h1, c1, h2, c2, h3, c3, h4, c4]
NCORES = 8
NT = 17           # tiles per core (2056 rows -> 17*128 padded)
OUT_NAMES = ["h1", "c1", "h2", "c2", "h3", "c3", "h4", "c4"]
BF = ml_dtypes.bfloat16


# ---------------------------------------------------------------- host plan

def _make_plan(batch_sizes):
    bs = np.asarray(batch_sizes).astype(np.int64)
    s = np.concatenate([i * b + np.arange(b) for i, b in enumerate(bs)]).astype(np.int64)
    U = int(s.max()) + 1
    n_rows = (U + NCORES - 1) // NCORES           # rows per core (max)
    nt = (n_rows + P - 1) // P
    return dict(s=s, Nout=int(s.size), U=U, n_rows=n_rows, nt=nt)


def _pack_weights(inputs):
    """-> w [128, 16*G] bf16 (per (layer, kchunk): rows of W^T igo),
          b [128, 4*G] bf16 (broadcast bias)."""
    w = np.zeros((P, 16 * G), BF)
    b = np.zeros((P, 4 * G), BF)
    for li in range(4):
        W = np.asarray(inputs[f"W_ih{li+1}"], np.float32)        # [2048, 512]
        bb = (np.asarray(inputs[f"b_ih{li+1}"], np.float32)
              + np.asarray(inputs[f"b_hh{li+1}"], np.float32))   # [2048]
        Wigo = np.concatenate([W[0:H], W[3*H:4*H], W[2*H:3*H]], axis=0)
        bigo = np.concatenate([bb[0:H], bb[3*H:4*H], bb[2*H:3*H]])
        WT = Wigo.T                                              # [512, 1536]
        for k in range(4):
            w[:, (li * 4 + k) * G:(li * 4 + k + 1) * G] = \
                WT[k * P:(k + 1) * P].astype(BF)
        b[:, li * G:(li + 1) * G] = np.broadcast_to(
            bigo.astype(BF)[None, :], (P, G))
    return w, b


# ---------------------------------------------------------------- bass build

def _build_nc(nt, wide_mm=True):
    import concourse.mybir as mybir
    from concourse import bacc
    from concourse.masks import make_identity
    from concourse.tile import TileContext

    dt = mybir.dt
    AF = mybir.ActivationFunctionType
    OP = mybir.AluOpType

    nc = bacc.Bacc()
    # x pre-transposed on host: [feat_in_chunk(128), chunk(4), tile(nt), tok(128)]
    x_d = nc.dram_tensor("x", [P, 4 * nt * P], dt.bfloat16, kind="ExternalInput")
    w_d = nc.dram_tensor("w", [P, 16 * G], dt.bfloat16, kind="ExternalInput")
    b_d = nc.dram_tensor("b", [P, 4 * G], dt.bfloat16, kind="ExternalInput")
    o_d = nc.dram_tensor("hc", [nt * P, J * H], dt.bfloat16, kind="ExternalOutput")

    with TileContext(nc) as tc:
        with (
            tc.tile_pool(name="const", bufs=1) as constp,
            tc.tile_pool(name="aT", bufs=4) as aTp,
            tc.tile_pool(name="gsb", bufs=4) as gsbp,
            tc.tile_pool(name="tt", bufs=4) as ttp,
            tc.tile_pool(name="tc2", bufs=4) as tcp,
            tc.tile_pool(name="hc", bufs=6) as hcp,
            tc.tile_pool(name="psg", bufs=2, space="PSUM") as psgp,
        ):
            # Separate per-layer weight/bias tiles so the first wave only
            # waits for x + layer-0 weights (startup was DMA-bandwidth bound).
            x_sb = constp.tile([P, 4 * nt * P], dt.bfloat16)
            nc.gpsimd.dma_start(x_sb[:], x_d[:])
            w_sbs, b_sbs = [], []
            for li in range(4):
                w_li = constp.tile([P, 4 * G], dt.bfloat16, name=f"w{li}")
                b_li = constp.tile([P, G], dt.bfloat16, name=f"bb{li}")
                q = nc.gpsimd if li == 0 else nc.sync
                q.dma_start(w_li[:], w_d[:, li * 4 * G:(li + 1) * 4 * G])
                q.dma_start(b_li[:], b_d[:, li * G:(li + 1) * G])
                w_sbs.append(w_li)
                b_sbs.append(b_li)

            # Software-pipelined wave schedule: wave w advances 4 independent
            # tile chains one layer each: jobs (w,0) (w-1,1) (w-2,2) (w-3,3).
            # Emission order fixes per-engine in-order streams to avoid
            # head-of-line blocking (each engine always has ready work).
            state = [None] * nt

            def emit_xpose(t, li):
                # xbar DMA transpose (SBUF->SBUF, bf16): aT chunks off PE/DVE
                st = state[t]
                h_prev = st["hc"][:, (2 * (li - 1)) * H:(2 * (li - 1) + 1) * H]
                aT = aTp.tile([P, H], dt.bfloat16, name="aT", tag="aT")
                for k in range(4):
                    nc.sync.dma_start_transpose(
                        aT[:, k * P:(k + 1) * P],
                        h_prev[:, k * P:(k + 1) * P])
                st["aT"] = aT

            def emit_mm(t, li):
                if li == 0:
                    state[t] = {"hc": hcp.tile([P, J * H], dt.bfloat16,
                                               name="hc", tag="hc")}
                st = state[t]
                g_ps = psgp.tile([P, G], dt.float32, tag="psg")
                for k in range(4):
                    if li == 0:
                        lhsT = x_sb[:, (k * nt + t) * P:(k * nt + t + 1) * P]
                    else:
                        lhsT = st["aT"][:, k * P:(k + 1) * P]
                    wbase = k * G
                    for n in range(3):
                        nc.tensor.matmul(
                            g_ps[:, n * H:(n + 1) * H],
                            lhsT,
                            w_sbs[li][:, wbase + n * H:wbase + (n + 1) * H],
                            start=(k == 0), stop=(k == 3))
                st["g_ps"] = g_ps

            def emit_bias(t, li):
                st = state[t]
                g_sb = gsbp.tile([P, G], dt.bfloat16, tag="gsb")
                nc.vector.tensor_add(g_sb[:], st["g_ps"][:], b_sbs[li][:])
                st["g_sb"] = g_sb

            def emit_tanh(t):
                # sigmoid over [i,o] (1024) + tanh over [g] (512); same
                # ACT table set, so no table reload between them.
                st = state[t]
                t_sb = ttp.tile([P, G], dt.bfloat16, name="t_sb", tag="tt")
                nc.scalar.activation(t_sb[:, 0:2 * H], st["g_sb"][:, 0:2 * H],
                                     AF.Sigmoid)
                nc.scalar.activation(t_sb[:, 2 * H:G], st["g_sb"][:, 2 * H:G],
                                     AF.Tanh)
                st["t_sb"] = t_sb

            def emit_sttc(t, li):
                # c = sigmoid(i) * tanh(g)
                st = state[t]
                c_t = st["hc"][:, (2 * li + 1) * H:(2 * li + 2) * H]
                nc.gpsimd.tensor_mul(c_t, st["t_sb"][:, 0:H],
                                     st["t_sb"][:, 2 * H:G])

            def emit_tanhc(t, li):
                st = state[t]
                c_t = st["hc"][:, (2 * li + 1) * H:(2 * li + 2) * H]
                tc_sb = tcp.tile([P, H], dt.bfloat16, name="tc_sb", tag="tc")
                nc.scalar.activation(tc_sb[:], c_t, AF.Tanh)
                st["tc"] = tc_sb

            def emit_stth(t, li):
                # h = sigmoid(o) * tanh(c)
                st = state[t]
                h_t = st["hc"][:, (2 * li) * H:(2 * li + 1) * H]
                nc.gpsimd.tensor_mul(h_t, st["t_sb"][:, H:2 * H], st["tc"][:])

            for w in range(nt + 3):
                jb = {l: w - l for l in range(4) if 0 <= w - l < nt}
                # PE stream:  mm0, mm1, mm2, mm3 (nothing else on PE)
                # DVE stream: bias0..bias3 (nothing else on DVE)
                # SYNC queue: 12 xbar transposes + the store
                for l in (1, 2, 3):
                    if l in jb:
                        emit_xpose(jb[l], l)
                for l in range(4):
                    if l in jb:
                        emit_mm(jb[l], l)
                        emit_bias(jb[l], l)
                # ACT stream: th0, th1, thc0, th2, thc1, th3, thc2, thc3
                # GPS stream: sc0, sc1, sh0, sc2, sh1, sc3, sh2, sh3
                if 0 in jb:
                    emit_tanh(jb[0])
                if 1 in jb:
                    emit_tanh(jb[1])
                if 0 in jb:
                    emit_sttc(jb[0], 0)
                    emit_tanhc(jb[0], 0)
                if 1 in jb:
                    emit_sttc(jb[1], 1)
                if 2 in jb:
                    emit_tanh(jb[2])
                if 0 in jb:
                    emit_stth(jb[0], 0)
                if 1 in jb:
                    emit_tanhc(jb[1], 1)
                if 2 in jb:
                    emit_sttc(jb[2], 2)
                if 3 in jb:
                    emit_tanh(jb[3])
                if 1 in jb:
                    emit_stth(jb[1], 1)
                if 2 in jb:
                    emit_tanhc(jb[2], 2)
                if 3 in jb:
                    emit_sttc(jb[3], 3)
                if 2 in jb:
                    emit_stth(jb[2], 2)
                if 3 in jb:
                    emit_tanhc(jb[3], 3)
                    emit_stth(jb[3], 3)
                    t3 = jb[3]
                    nc.sync.dma_start(o_d[t3 * P:(t3 + 1) * P, :],
                                      state[t3]["hc"][:])
    nc.compile()
    return nc


# ---------------------------------------------------------------- entry point

def _ensure_axon_hooks():
    """bass_utils' trace path imports antenv.axon_hooks, which some images
    lack; install a shim that drives NTFF profiling via libaxon_pjrt.so
    (mirrors the boot-side _ntff_profile_via_ctypes) or degrades to None."""
    try:
        import antenv.axon_hooks  # noqa: F401
        return
    except ImportError:
        pass
    import types
    import contextlib
    import ctypes

    def _build_hook():
        so = "/opt/axon/libaxon_pjrt.so"
        try:
            lib = ctypes.CDLL(so)
        except OSError:
            return None
        if not hasattr(lib, "axon_start_nrt_profile"):
            return None
        lib.axon_start_nrt_profile.argtypes = [
            ctypes.POINTER(ctypes.c_int64), ctypes.c_size_t]
        lib.axon_start_nrt_profile.restype = ctypes.c_int64
        lib.axon_stop_nrt_profile.argtypes = [ctypes.c_char_p]
        lib.axon_stop_nrt_profile.restype = ctypes.c_int64

        @contextlib.contextmanager
        def _hook(output_dir, device_ids):
            import jax
            jax.devices()
            if device_ids:
                ids = (ctypes.c_int64 * len(device_ids))(*device_ids)
                rc = lib.axon_start_nrt_profile(ids, len(device_ids))
            else:
                rc = lib.axon_start_nrt_profile(None, 0)
            if rc != 0:
                raise RuntimeError(f"axon_start_nrt_profile rc={rc}")
            try:
                yield
            finally:
                n = lib.axon_stop_nrt_profile(str(output_dir).encode())
                print(f"ntff profile: {n} file(s) written to {output_dir}",
                      file=sys.stderr)

        return _hook

    box = [None, False]

    def set_axon_ntff_profile_hook(h):
        box[0] = h
        box[1] = True

    def get_axon_ntff_profile_hook():
        if not box[1]:
            box[0] = _build_hook()
            box[1] = True
        return box[0]

    mod = types.ModuleType("antenv.axon_hooks")
    mod.set_axon_ntff_profile_hook = set_axon_ntff_profile_hook
    mod.get_axon_ntff_profile_hook = get_axon_ntff_profile_hook
    import antenv
    sys.modules["antenv.axon_hooks"] = mod
    antenv.axon_hooks = mod


_cache = {}


def kernel(**inputs):
    packed_x = np.asarray(inputs["packed_x"], np.float32)
    bs = np.asarray(inputs["batch_sizes"])

    key = bs.tobytes()
    if key not in _cache:
        plan = _make_plan(bs)
        # N=1536 matmuls (3 PSUM banks) fail the neuronxcc ISA check
        # (NCC_IXCG864); N=512 (one bank) is the legal max.
        nc = _build_nc(plan["nt"], wide_mm=False)
        _cache[key] = (plan, nc)
    plan, nc = _cache[key]

    w, b = _pack_weights(inputs)
    nt, U = plan["nt"], plan["U"]

    in_maps = []
    for c in range(NCORES):
        rows = np.arange(c, U, NCORES)
        xc = np.zeros((nt * P, H), np.float32)
        xc[:len(rows)] = packed_x[rows]
        # [tile, tok, chunk, feat] -> [feat, chunk, tile, tok]
        xT = np.ascontiguousarray(
            xc.reshape(nt, P, 4, P).transpose(3, 2, 0, 1)
        ).reshape(P, 4 * nt * P).astype(BF)
        in_maps.append({"x": xT, "w": w, "b": b})

    from concourse.bass_utils import run_bass_kernel_spmd
    _ensure_axon_hooks()
    res = run_bass_kernel_spmd(nc, in_maps, core_ids=list(range(NCORES)))
    global last_result
    last_result = res

    s = plan["s"]
    idx_c = s % NCORES
    idx_p = s // NCORES
    # [8, nt*P, J*H] device outputs (bf16 h/c per layer)
    slabs = np.stack([np.asarray(res.results[c]["hc"]) for c in range(NCORES)])
    full = {}
    for jo, nm in enumerate(OUT_NAMES):
        full[nm] = slabs[idx_c, idx_p, jo * H:(jo + 1) * H].astype(np.float32)

    return (full["h4"], full["h1"], full["c1"], full["h2"], full["c2"],
            full["h3"], full["c3"], full["h4"], full["c4"])


if __name__ == "__main__":
    import reference
    inputs = reference.setup_inputs()
    out = kernel(**{k: np.asarray(v) for k, v in inputs.items()})
    print([o.shape for o in out])


# revision 19
# speedup vs baseline: 1.4020x; 1.4020x over previous
"""Trainium2 Bass kernel for nn_Encoder_LSTM (4x LSTMCell with zero state over
packed ragged tokens).

Math (from the reference): all rows independent; for each output row j with
source row s(j) (the ragged gather), and each of 4 layers:
    gates = x @ W_ih^T + (b_ih + b_hh);  i, f, g, o = split(gates)
    c = sigmoid(i) * tanh(g);  h = sigmoid(o) * tanh(c)      (f is unused)
Outputs: (output=h4, h1, c1, h2, c2, h3, c3, h4, c4), each [sum(bs), 512] fp32.

v2 strategy (vs the v1 slab kernel):
  - Compute each of the U=16448 distinct source rows once; core c takes rows
    c::8 (2056 rows = 17 tiles of 128). Store ONLY distinct rows, in bf16;
    the host expands duplicates + upcasts to f32.
  - tanh-only activation path: sigmoid(z) = (tanh(z/2)+1)/2. The i/o gate
    weight+bias columns are pre-scaled by 0.5 on the host so ONE tanh over
    all 1536 packed gates [i,o,g] gives t_i, t_o, t_g. Then
        c_raw = (t_i + 1) * t_g          ( = 2c )
        h_raw = (t_o + 1) * tanh(0.5*c_raw)   ( = 2h )
    each ONE fused DVE scalar_tensor_tensor in bf16 (2x mode). The factor 2
    on h is folded into the next layer's weights (x0.5), and the stored
    h_raw/c_raw are halved on the host.
  - Host pre-transposes x per core, so layer-1 lhsT slices come straight from
    SBUF. Inter-layer transposes are REGULAR matmuls against a bf16 identity
    (fast warm-PE path, f32 PSUM out); ACT copies PSUM -> bf16 SBUF.
  - Bias add (free-dim varying, so not expressible as ACT per-partition
    bias) is one DVE tensor_tensor from PSUM per layer.
  - Stores: one [128, 8*512] bf16 DMA per tile (8KB/partition contiguous).
"""

import sys

if "/opt/trn_rl_repo" not in sys.path:
    sys.path.insert(0, "/opt/trn_rl_repo")

import numpy as np
import ml_dtypes

P = 128
H = 512
G = 1536          # 3 packed gates [i, o, g] * 512
J = 8             # fused outputs [h1, c1, h2, c2, h3, c3, h4, c4]
NCORES = 8
NT = 17           # tiles per core (2056 rows -> 17*128 padded)
OUT_NAMES = ["h1", "c1", "h2", "c2", "h3", "c3", "h4", "c4"]
BF = ml_dtypes.bfloat16


# ---------------------------------------------------------------- host plan

def _make_plan(batch_sizes):
    bs = np.asarray(batch_sizes).astype(np.int64)
    s = np.concatenate([i * b + np.arange(b) for i, b in enumerate(bs)]).astype(np.int64)
    U = int(s.max()) + 1
    n_rows = (U + NCORES - 1) // NCORES           # rows per core (max)
    nt = (n_rows + P - 1) // P
    return dict(s=s, Nout=int(s.size), U=U, n_rows=n_rows, nt=nt)


def _pack_weights(inputs):
    """-> w [128, 16*G] bf16 (per (layer, kchunk): rows of W^T igo),
          b [128, 4*G] bf16 (broadcast bias)."""
    w = np.zeros((P, 16 * G), BF)
    b = np.zeros((P, 4 * G), BF)
    for li in range(4):
        W = np.asarray(inputs[f"W_ih{li+1}"], np.float32)        # [2048, 512]
        bb = (np.asarray(inputs[f"b_ih{li+1}"], np.float32)
              + np.asarray(inputs[f"b_hh{li+1}"], np.float32))   # [2048]
        Wigo = np.concatenate([W[0:H], W[3*H:4*H], W[2*H:3*H]], axis=0)
        bigo = np.concatenate([bb[0:H], bb[3*H:4*H], bb[2*H:3*H]])
        WT = Wigo.T                                              # [512, 1536]
        for k in range(4):
            w[:, (li * 4 + k) * G:(li * 4 + k + 1) * G] = \
                WT[k * P:(k + 1) * P].astype(BF)
        b[:, li * G:(li + 1) * G] = np.broadcast_to(
            bigo.astype(BF)[None, :], (P, G))
    return w, b


# ---------------------------------------------------------------- bass build

def _build_nc(nt, wide_mm=True):
    import concourse.mybir as mybir
    from concourse import bacc
    from concourse.masks import make_identity
    from concourse.tile import TileContext

    dt = mybir.dt
    AF = mybir.ActivationFunctionType
    OP = mybir.AluOpType

    nc = bacc.Bacc()
    # x pre-transposed on host: [feat_in_chunk(128), chunk(4), tile(nt), tok(128)]
    x_d = nc.dram_tensor("x", [P, 4 * nt * P], dt.bfloat16, kind="ExternalInput")
    w_d = nc.dram_tensor("w", [P, 16 * G], dt.bfloat16, kind="ExternalInput")
    b_d = nc.dram_tensor("b", [P, 4 * G], dt.bfloat16, kind="ExternalInput")
    o_d = nc.dram_tensor("hc", [nt * P, J * H], dt.bfloat16, kind="ExternalOutput")

    with TileContext(nc) as tc:
        with (
            tc.tile_pool(name="const", bufs=1) as constp,
            tc.tile_pool(name="aT", bufs=4) as aTp,
            tc.tile_pool(name="gsb", bufs=4) as gsbp,
            tc.tile_pool(name="tt", bufs=4) as ttp,
            tc.tile_pool(name="tc2", bufs=4) as tcp,
            tc.tile_pool(name="hc", bufs=6) as hcp,
            tc.tile_pool(name="psg", bufs=2, space="PSUM") as psgp,
        ):
            # Separate per-layer weight/bias tiles so the first wave only
            # waits for x + layer-0 weights (startup was DMA-bandwidth bound).
            x_sb = constp.tile([P, 4 * nt * P], dt.bfloat16)
            nc.gpsimd.dma_start(x_sb[:], x_d[:])
            w_sbs, b_sbs = [], []
            for li in range(4):
                w_li = constp.tile([P, 4 * G], dt.bfloat16, name=f"w{li}")
                b_li = constp.tile([P, G], dt.bfloat16, name=f"bb{li}")
                q = nc.gpsimd if li == 0 else nc.sync
                q.dma_start(w_li[:], w_d[:, li * 4 * G:(li + 1) * 4 * G])
                q.dma_start(b_li[:], b_d[:, li * G:(li + 1) * G])
                w_sbs.append(w_li)
                b_sbs.append(b_li)

            # Software-pipelined wave schedule: wave w advances 4 independent
            # tile chains one layer each: jobs (w,0) (w-1,1) (w-2,2) (w-3,3).
            # Emission order fixes per-engine in-order streams to avoid
            # head-of-line blocking (each engine always has ready work).
            state = [None] * nt

            def emit_xpose(t, li):
                # xbar DMA transpose (SBUF->SBUF, bf16): aT chunks off PE/DVE.
                # One instruction transposes all 4 chunks: out[p, k, j] =
                # h[j, k*128+p] (16x128 xbar tiles land per-chunk).
                st = state[t]
                h_prev = st["hc"][:, (2 * (li - 1)) * H:(2 * (li - 1) + 1) * H]
                aT = aTp.tile([P, H], dt.bfloat16, name="aT", tag="aT")
                nc.sync.dma_start_transpose(
                    aT[:].rearrange("p (k j) -> p k j", j=P), h_prev)
                st["aT"] = aT

            def emit_mm(t, li):
                if li == 0:
                    state[t] = {"hc": hcp.tile([P, J * H], dt.bfloat16,
                                               name="hc", tag="hc")}
                st = state[t]
                g_ps = psgp.tile([P, G], dt.float32, tag="psg")
                for k in range(4):
                    if li == 0:
                        lhsT = x_sb[:, (k * nt + t) * P:(k * nt + t + 1) * P]
                    else:
                        lhsT = st["aT"][:, k * P:(k + 1) * P]
                    wbase = k * G
                    for n in range(3):
                        nc.tensor.matmul(
                            g_ps[:, n * H:(n + 1) * H],
                            lhsT,
                            w_sbs[li][:, wbase + n * H:wbase + (n + 1) * H],
                            start=(k == 0), stop=(k == 3))
                st["g_ps"] = g_ps

            def emit_bias(t, li):
                st = state[t]
                g_sb = gsbp.tile([P, G], dt.bfloat16, tag="gsb")
                nc.vector.tensor_add(g_sb[:], st["g_ps"][:], b_sbs[li][:])
                st["g_sb"] = g_sb

            def emit_tanh(t):
                # sigmoid over [i,o] (1024) + tanh over [g] (512); same
                # ACT table set, so no table reload between them.
                st = state[t]
                t_sb = ttp.tile([P, G], dt.bfloat16, name="t_sb", tag="tt")
                nc.scalar.activation(t_sb[:, 0:2 * H], st["g_sb"][:, 0:2 * H],
                                     AF.Sigmoid)
                nc.scalar.activation(t_sb[:, 2 * H:G], st["g_sb"][:, 2 * H:G],
                                     AF.Tanh)
                st["t_sb"] = t_sb

            def emit_sttc(t, li):
                # c = sigmoid(i) * tanh(g)
                st = state[t]
                c_t = st["hc"][:, (2 * li + 1) * H:(2 * li + 2) * H]
                nc.gpsimd.tensor_mul(c_t, st["t_sb"][:, 0:H],
                                     st["t_sb"][:, 2 * H:G])

            def emit_tanhc(t, li):
                st = state[t]
                c_t = st["hc"][:, (2 * li + 1) * H:(2 * li + 2) * H]
                tc_sb = tcp.tile([P, H], dt.bfloat16, name="tc_sb", tag="tc")
                nc.scalar.activation(tc_sb[:], c_t, AF.Tanh)
                st["tc"] = tc_sb

            def emit_stth(t, li):
                # h = sigmoid(o) * tanh(c)
                st = state[t]
                h_t = st["hc"][:, (2 * li) * H:(2 * li + 1) * H]
                nc.gpsimd.tensor_mul(h_t, st["t_sb"][:, H:2 * H], st["tc"][:])

            for w in range(nt + 3):
                jb = {l: w - l for l in range(4) if 0 <= w - l < nt}
                # PE stream:  mm0, mm1, mm2, mm3 (nothing else on PE)
                # DVE stream: bias0..bias3 (nothing else on DVE)
                # SYNC queue: 12 xbar transposes + the store
                for l in (1, 2, 3):
                    if l in jb:
                        emit_xpose(jb[l], l)
                for l in range(4):
                    if l in jb:
                        emit_mm(jb[l], l)
                        emit_bias(jb[l], l)
                # ACT stream: th0, th1, thc0, th2, thc1, th3, thc2, thc3
                # GPS stream: sc0, sc1, sh0, sc2, sh1, sc3, sh2, sh3
                if 0 in jb:
                    emit_tanh(jb[0])
                if 1 in jb:
                    emit_tanh(jb[1])
                if 0 in jb:
                    emit_sttc(jb[0], 0)
                    emit_tanhc(jb[0], 0)
                if 1 in jb:
                    emit_sttc(jb[1], 1)
                if 2 in jb:
                    emit_tanh(jb[2])
                if 0 in jb:
                    emit_stth(jb[0], 0)
                if 1 in jb:
                    emit_tanhc(jb[1], 1)
                if 2 in jb:
                    emit_sttc(jb[2], 2)
                if 3 in jb:
                    emit_tanh(jb[3])
                if 1 in jb:
                    emit_stth(jb[1], 1)
                if 2 in jb:
                    emit_tanhc(jb[2], 2)
                if 3 in jb:
                    emit_sttc(jb[3], 3)
                if 2 in jb:
                    emit_stth(jb[2], 2)
                if 3 in jb:
                    emit_tanhc(jb[3], 3)
                    emit_stth(jb[3], 3)
                    t3 = jb[3]
                    nc.sync.dma_start(o_d[t3 * P:(t3 + 1) * P, :],
                                      state[t3]["hc"][:])
    nc.compile()
    return nc


# ---------------------------------------------------------------- entry point

def _ensure_axon_hooks():
    """bass_utils' trace path imports antenv.axon_hooks, which some images
    lack; install a shim that drives NTFF profiling via libaxon_pjrt.so
    (mirrors the boot-side _ntff_profile_via_ctypes) or degrades to None."""
    try:
        import antenv.axon_hooks  # noqa: F401
        return
    except ImportError:
        pass
    import types
    import contextlib
    import ctypes

    def _build_hook():
        so = "/opt/axon/libaxon_pjrt.so"
        try:
            lib = ctypes.CDLL(so)
        except OSError:
            return None
        if not hasattr(lib, "axon_start_nrt_profile"):
            return None
        lib.axon_start_nrt_profile.argtypes = [
            ctypes.POINTER(ctypes.c_int64), ctypes.c_size_t]
        lib.axon_start_nrt_profile.restype = ctypes.c_int64
        lib.axon_stop_nrt_profile.argtypes = [ctypes.c_char_p]
        lib.axon_stop_nrt_profile.restype = ctypes.c_int64

        @contextlib.contextmanager
        def _hook(output_dir, device_ids):
            import jax
            jax.devices()
            if device_ids:
                ids = (ctypes.c_int64 * len(device_ids))(*device_ids)
                rc = lib.axon_start_nrt_profile(ids, len(device_ids))
            else:
                rc = lib.axon_start_nrt_profile(None, 0)
            if rc != 0:
                raise RuntimeError(f"axon_start_nrt_profile rc={rc}")
            try:
                yield
            finally:
                n = lib.axon_stop_nrt_profile(str(output_dir).encode())
                print(f"ntff profile: {n} file(s) written to {output_dir}",
                      file=sys.stderr)

        return _hook

    box = [None, False]

    def set_axon_ntff_profile_hook(h):
        box[0] = h
        box[1] = True

    def get_axon_ntff_profile_hook():
        if not box[1]:
            box[0] = _build_hook()
            box[1] = True
        return box[0]

    mod = types.ModuleType("antenv.axon_hooks")
    mod.set_axon_ntff_profile_hook = set_axon_ntff_profile_hook
    mod.get_axon_ntff_profile_hook = get_axon_ntff_profile_hook
    import antenv
    sys.modules["antenv.axon_hooks"] = mod
    antenv.axon_hooks = mod


_cache = {}


def kernel(**inputs):
    packed_x = np.asarray(inputs["packed_x"], np.float32)
    bs = np.asarray(inputs["batch_sizes"])

    key = bs.tobytes()
    if key not in _cache:
        plan = _make_plan(bs)
        # N=1536 matmuls (3 PSUM banks) fail the neuronxcc ISA check
        # (NCC_IXCG864); N=512 (one bank) is the legal max.
        nc = _build_nc(plan["nt"], wide_mm=False)
        _cache[key] = (plan, nc)
    plan, nc = _cache[key]

    w, b = _pack_weights(inputs)
    nt, U = plan["nt"], plan["U"]

    in_maps = []
    for c in range(NCORES):
        rows = np.arange(c, U, NCORES)
        xc = np.zeros((nt * P, H), np.float32)
        xc[:len(rows)] = packed_x[rows]
        # [tile, tok, chunk, feat] -> [feat, chunk, tile, tok]
        xT = np.ascontiguousarray(
            xc.reshape(nt, P, 4, P).transpose(3, 2, 0, 1)
        ).reshape(P, 4 * nt * P).astype(BF)
        in_maps.append({"x": xT, "w": w, "b": b})

    from concourse.bass_utils import run_bass_kernel_spmd
    _ensure_axon_hooks()
    res = run_bass_kernel_spmd(nc, in_maps, core_ids=list(range(NCORES)))
    global last_result
    last_result = res

    s = plan["s"]
    idx_c = s % NCORES
    idx_p = s // NCORES
    # [8, nt*P, J*H] device outputs (bf16 h/c per layer)
    slabs = np.stack([np.asarray(res.results[c]["hc"]) for c in range(NCORES)])
    full = {}
    for jo, nm in enumerate(OUT_NAMES):
        full[nm] = slabs[idx_c, idx_p, jo * H:(jo + 1) * H].astype(np.float32)

    return (full["h4"], full["h1"], full["c1"], full["h2"], full["c2"],
            full["h3"], full["c3"], full["h4"], full["c4"])


if __name__ == "__main__":
    import reference
    inputs = reference.setup_inputs()
    out = kernel(**{k: np.asarray(v) for k, v in inputs.items()})
    print([o.shape for o in out])


# revision 22
# speedup vs baseline: 1.7065x; 1.2172x over previous
"""Trainium2 Bass kernel for nn_Encoder_LSTM (4x LSTMCell with zero state over
packed ragged tokens).

Math (from the reference): all rows independent; for each output row j with
source row s(j) (the ragged gather), and each of 4 layers:
    gates = x @ W_ih^T + (b_ih + b_hh);  i, f, g, o = split(gates)
    c = sigmoid(i) * tanh(g);  h = sigmoid(o) * tanh(c)      (f is unused)
Outputs: (output=h4, h1, c1, h2, c2, h3, c3, h4, c4), each [sum(bs), 512] fp32.

v2 strategy (vs the v1 slab kernel):
  - Compute each of the U=16448 distinct source rows once; core c takes rows
    c::8 (2056 rows = 17 tiles of 128). Store ONLY distinct rows, in bf16;
    the host expands duplicates + upcasts to f32.
  - tanh-only activation path: sigmoid(z) = (tanh(z/2)+1)/2. The i/o gate
    weight+bias columns are pre-scaled by 0.5 on the host so ONE tanh over
    all 1536 packed gates [i,o,g] gives t_i, t_o, t_g. Then
        c_raw = (t_i + 1) * t_g          ( = 2c )
        h_raw = (t_o + 1) * tanh(0.5*c_raw)   ( = 2h )
    each ONE fused DVE scalar_tensor_tensor in bf16 (2x mode). The factor 2
    on h is folded into the next layer's weights (x0.5), and the stored
    h_raw/c_raw are halved on the host.
  - Host pre-transposes x per core, so layer-1 lhsT slices come straight from
    SBUF. Inter-layer transposes are REGULAR matmuls against a bf16 identity
    (fast warm-PE path, f32 PSUM out); ACT copies PSUM -> bf16 SBUF.
  - Bias add (free-dim varying, so not expressible as ACT per-partition
    bias) is one DVE tensor_tensor from PSUM per layer.
  - Stores: one [128, 8*512] bf16 DMA per tile (8KB/partition contiguous).
"""

import sys

if "/opt/trn_rl_repo" not in sys.path:
    sys.path.insert(0, "/opt/trn_rl_repo")

import numpy as np
import ml_dtypes

P = 128
H = 512
G = 1536          # 3 packed gates [i, o, g] * 512
J = 8             # fused outputs [h1, c1, h2, c2, h3, c3, h4, c4]
NCORES = 8
NT = 17           # tiles per core (2056 rows -> 17*128 padded)
OUT_NAMES = ["h1", "c1", "h2", "c2", "h3", "c3", "h4", "c4"]
BF = ml_dtypes.bfloat16


# ---------------------------------------------------------------- host plan

def _make_plan(batch_sizes):
    bs = np.asarray(batch_sizes).astype(np.int64)
    s = np.concatenate([i * b + np.arange(b) for i, b in enumerate(bs)]).astype(np.int64)
    U = int(s.max()) + 1
    n_rows = (U + NCORES - 1) // NCORES           # rows per core (max)
    nt = (n_rows + P - 1) // P
    return dict(s=s, Nout=int(s.size), U=U, n_rows=n_rows, nt=nt)


def _pack_weights(inputs):
    """-> w [128, 16*G] bf16 (per (layer, kchunk): rows of W^T igo),
          b [128, 4*G] bf16 (broadcast bias)."""
    w = np.zeros((P, 16 * G), BF)
    b = np.zeros((P, 4 * G), BF)
    for li in range(4):
        W = np.asarray(inputs[f"W_ih{li+1}"], np.float32)        # [2048, 512]
        bb = (np.asarray(inputs[f"b_ih{li+1}"], np.float32)
              + np.asarray(inputs[f"b_hh{li+1}"], np.float32))   # [2048]
        Wigo = np.concatenate([W[0:H], W[3*H:4*H], W[2*H:3*H]], axis=0)
        bigo = np.concatenate([bb[0:H], bb[3*H:4*H], bb[2*H:3*H]])
        WT = Wigo.T                                              # [512, 1536]
        for k in range(4):
            w[:, (li * 4 + k) * G:(li * 4 + k + 1) * G] = \
                WT[k * P:(k + 1) * P].astype(BF)
        b[:, li * G:(li + 1) * G] = np.broadcast_to(
            bigo.astype(BF)[None, :], (P, G))
    return w, b


# ---------------------------------------------------------------- bass build

def _build_nc(nt, wide_mm=True):
    import concourse.mybir as mybir
    from concourse import bacc
    from concourse.masks import make_identity
    from concourse.tile import TileContext

    dt = mybir.dt
    AF = mybir.ActivationFunctionType
    OP = mybir.AluOpType

    nc = bacc.Bacc()
    # x pre-transposed on host: [feat_in_chunk(128), chunk(4), tile(nt), tok(128)]
    x_d = nc.dram_tensor("x", [P, 4 * nt * P], dt.bfloat16, kind="ExternalInput")
    w_d = nc.dram_tensor("w", [P, 16 * G], dt.bfloat16, kind="ExternalInput")
    b_d = nc.dram_tensor("b", [P, 4 * G], dt.bfloat16, kind="ExternalInput")
    o_d = nc.dram_tensor("hc", [nt * P, J * H], dt.bfloat16, kind="ExternalOutput")

    with TileContext(nc) as tc:
        with (
            tc.tile_pool(name="const", bufs=1) as constp,
            tc.tile_pool(name="aT", bufs=4) as aTp,
            tc.tile_pool(name="gsb", bufs=5) as gsbp,
            tc.tile_pool(name="tt", bufs=5) as ttp,
            tc.tile_pool(name="tc2", bufs=6) as tcp,
            tc.tile_pool(name="hc", bufs=8) as hcp,
            tc.tile_pool(name="psg", bufs=2, space="PSUM") as psgp,
        ):
            # Startup is DMA-bound: spread the input loads over all three DMA
            # issue queues (sync/scalar HWDGE + gpsimd SWDGE), critical bytes
            # first (x halves + layer-0 weights), so wave 0 starts ~10us in.
            x_sb = constp.tile([P, 4 * nt * P], dt.bfloat16)
            w_sbs = [constp.tile([P, 4 * G], dt.bfloat16, name=f"w{li}")
                     for li in range(4)]
            b_sbs = [constp.tile([P, G], dt.bfloat16, name=f"bb{li}")
                     for li in range(4)]
            queues = [nc.sync, nc.scalar, nc.gpsimd]
            half = (nt + 1) // 2
            for k in range(4):
                queues[k % 3].dma_start(
                    x_sb[:, (k * nt) * P:(k * nt + half) * P],
                    x_d[:, (k * nt) * P:(k * nt + half) * P])
            for k in range(4):
                queues[(k + 1) % 3].dma_start(
                    w_sbs[0][:, k * G:(k + 1) * G],
                    w_d[:, k * G:(k + 1) * G])
            nc.gpsimd.dma_start(b_sbs[0][:], b_d[:, 0:G])
            for k in range(4):
                queues[k % 3].dma_start(
                    x_sb[:, (k * nt + half) * P:(k * nt + nt) * P],
                    x_d[:, (k * nt + half) * P:(k * nt + nt) * P])
            for li, q in ((1, nc.scalar), (2, nc.gpsimd), (3, nc.sync)):
                q.dma_start(w_sbs[li][:], w_d[:, li * 4 * G:(li + 1) * 4 * G])
                q.dma_start(b_sbs[li][:], b_d[:, li * G:(li + 1) * G])

            # Software-pipelined wave schedule: wave w advances 4 independent
            # tile chains one layer each: jobs (w,0) (w-1,1) (w-2,2) (w-3,3).
            # Emission order fixes per-engine in-order streams to avoid
            # head-of-line blocking (each engine always has ready work).
            state = [None] * nt

            def emit_xpose(t, li):
                # xbar DMA transpose (SBUF->SBUF, bf16): aT chunks off PE/DVE.
                # One instruction transposes all 4 chunks: out[p, k, j] =
                # h[j, k*128+p] (16x128 xbar tiles land per-chunk).
                st = state[t]
                h_prev = st["hc"][:, (2 * (li - 1)) * H:(2 * (li - 1) + 1) * H]
                aT = aTp.tile([P, H], dt.bfloat16, name="aT", tag="aT")
                nc.sync.dma_start_transpose(
                    aT[:].rearrange("p (k j) -> p k j", j=P), h_prev)
                st["aT"] = aT

            def emit_mm(t, li):
                if li == 0:
                    state[t] = {"hc": hcp.tile([P, J * H], dt.bfloat16,
                                               name="hc", tag="hc")}
                st = state[t]
                g_ps = psgp.tile([P, G], dt.float32, tag="psg")
                for k in range(4):
                    if li == 0:
                        lhsT = x_sb[:, (k * nt + t) * P:(k * nt + t + 1) * P]
                    else:
                        lhsT = st["aT"][:, k * P:(k + 1) * P]
                    wbase = k * G
                    for n in range(3):
                        nc.tensor.matmul(
                            g_ps[:, n * H:(n + 1) * H],
                            lhsT,
                            w_sbs[li][:, wbase + n * H:wbase + (n + 1) * H],
                            start=(k == 0), stop=(k == 3))
                st["g_ps"] = g_ps

            def emit_bias(t, li):
                st = state[t]
                g_sb = gsbp.tile([P, G], dt.bfloat16, tag="gsb")
                nc.vector.tensor_add(g_sb[:], st["g_ps"][:], b_sbs[li][:])
                st["g_sb"] = g_sb

            def emit_tanh(t):
                # sigmoid over [i,o] (1024) + tanh over [g] (512); same
                # ACT table set, so no table reload between them.
                st = state[t]
                t_sb = ttp.tile([P, G], dt.bfloat16, name="t_sb", tag="tt")
                nc.scalar.activation(t_sb[:, 0:2 * H], st["g_sb"][:, 0:2 * H],
                                     AF.Sigmoid)
                nc.scalar.activation(t_sb[:, 2 * H:G], st["g_sb"][:, 2 * H:G],
                                     AF.Tanh)
                st["t_sb"] = t_sb

            def emit_sttc(t, li):
                # c = sigmoid(i) * tanh(g)
                st = state[t]
                c_t = st["hc"][:, (2 * li + 1) * H:(2 * li + 2) * H]
                nc.gpsimd.tensor_mul(c_t, st["t_sb"][:, 0:H],
                                     st["t_sb"][:, 2 * H:G])

            def emit_tanhc(t, li):
                st = state[t]
                c_t = st["hc"][:, (2 * li + 1) * H:(2 * li + 2) * H]
                tc_sb = tcp.tile([P, H], dt.bfloat16, name="tc_sb", tag="tc")
                nc.scalar.activation(tc_sb[:], c_t, AF.Tanh)
                st["tc"] = tc_sb

            def emit_stth(t, li):
                # h = sigmoid(o) * tanh(c)
                st = state[t]
                h_t = st["hc"][:, (2 * li) * H:(2 * li + 1) * H]
                nc.gpsimd.tensor_mul(h_t, st["t_sb"][:, H:2 * H], st["tc"][:])

            # 2-wave layer spacing: a tile advances one layer every TWO
            # waves, so each chain (bias->sigmoid/tanh->muls->transpose) has
            # ~2 wave periods of slack before its output feeds the next
            # matmul group -- the chain latency (~10us) then never stalls PE.
            for w in range(nt + 6):
                jb = {l: w - 2 * l for l in range(4) if 0 <= w - 2 * l < nt}
                # PE stream:  mm0, mm1, mm2, mm3 (nothing else on PE)
                # DVE stream: bias0..bias3 (nothing else on DVE)
                # SYNC queue: 12 xbar transposes + the store
                for l in (1, 2, 3):
                    if l in jb:
                        emit_xpose(jb[l], l)
                for l in range(4):
                    if l in jb:
                        emit_mm(jb[l], l)
                        emit_bias(jb[l], l)
                # ACT stream: th0, th1, thc0, th2, thc1, th3, thc2, thc3
                # GPS stream: sc0, sc1, sh0, sc2, sh1, sc3, sh2, sh3
                if 0 in jb:
                    emit_tanh(jb[0])
                if 1 in jb:
                    emit_tanh(jb[1])
                if 0 in jb:
                    emit_sttc(jb[0], 0)
                    emit_tanhc(jb[0], 0)
                if 1 in jb:
                    emit_sttc(jb[1], 1)
                if 2 in jb:
                    emit_tanh(jb[2])
                if 0 in jb:
                    emit_stth(jb[0], 0)
                if 1 in jb:
                    emit_tanhc(jb[1], 1)
                if 2 in jb:
                    emit_sttc(jb[2], 2)
                if 3 in jb:
                    emit_tanh(jb[3])
                if 1 in jb:
                    emit_stth(jb[1], 1)
                if 2 in jb:
                    emit_tanhc(jb[2], 2)
                if 3 in jb:
                    emit_sttc(jb[3], 3)
                if 2 in jb:
                    emit_stth(jb[2], 2)
                if 3 in jb:
                    emit_tanhc(jb[3], 3)
                    emit_stth(jb[3], 3)
                    t3 = jb[3]
                    nc.sync.dma_start(o_d[t3 * P:(t3 + 1) * P, :],
                                      state[t3]["hc"][:])
    nc.compile()
    return nc


# ---------------------------------------------------------------- entry point

def _ensure_axon_hooks():
    """bass_utils' trace path imports antenv.axon_hooks, which some images
    lack; install a shim that drives NTFF profiling via libaxon_pjrt.so
    (mirrors the boot-side _ntff_profile_via_ctypes) or degrades to None."""
    try:
        import antenv.axon_hooks  # noqa: F401
        return
    except ImportError:
        pass
    import types
    import contextlib
    import ctypes

    def _build_hook():
        so = "/opt/axon/libaxon_pjrt.so"
        try:
            lib = ctypes.CDLL(so)
        except OSError:
            return None
        if not hasattr(lib, "axon_start_nrt_profile"):
            return None
        lib.axon_start_nrt_profile.argtypes = [
            ctypes.POINTER(ctypes.c_int64), ctypes.c_size_t]
        lib.axon_start_nrt_profile.restype = ctypes.c_int64
        lib.axon_stop_nrt_profile.argtypes = [ctypes.c_char_p]
        lib.axon_stop_nrt_profile.restype = ctypes.c_int64

        @contextlib.contextmanager
        def _hook(output_dir, device_ids):
            import jax
            jax.devices()
            if device_ids:
                ids = (ctypes.c_int64 * len(device_ids))(*device_ids)
                rc = lib.axon_start_nrt_profile(ids, len(device_ids))
            else:
                rc = lib.axon_start_nrt_profile(None, 0)
            if rc != 0:
                raise RuntimeError(f"axon_start_nrt_profile rc={rc}")
            try:
                yield
            finally:
                n = lib.axon_stop_nrt_profile(str(output_dir).encode())
                print(f"ntff profile: {n} file(s) written to {output_dir}",
                      file=sys.stderr)

        return _hook

    box = [None, False]

    def set_axon_ntff_profile_hook(h):
        box[0] = h
        box[1] = True

    def get_axon_ntff_profile_hook():
        if not box[1]:
            box[0] = _build_hook()
            box[1] = True
        return box[0]

    mod = types.ModuleType("antenv.axon_hooks")
    mod.set_axon_ntff_profile_hook = set_axon_ntff_profile_hook
    mod.get_axon_ntff_profile_hook = get_axon_ntff_profile_hook
    import antenv
    sys.modules["antenv.axon_hooks"] = mod
    antenv.axon_hooks = mod


_cache = {}


def kernel(**inputs):
    packed_x = np.asarray(inputs["packed_x"], np.float32)
    bs = np.asarray(inputs["batch_sizes"])

    key = bs.tobytes()
    if key not in _cache:
        plan = _make_plan(bs)
        # N=1536 matmuls (3 PSUM banks) fail the neuronxcc ISA check
        # (NCC_IXCG864); N=512 (one bank) is the legal max.
        nc = _build_nc(plan["nt"], wide_mm=False)
        _cache[key] = (plan, nc)
    plan, nc = _cache[key]

    w, b = _pack_weights(inputs)
    nt, U = plan["nt"], plan["U"]

    in_maps = []
    for c in range(NCORES):
        rows = np.arange(c, U, NCORES)
        xc = np.zeros((nt * P, H), np.float32)
        xc[:len(rows)] = packed_x[rows]
        # [tile, tok, chunk, feat] -> [feat, chunk, tile, tok]
        xT = np.ascontiguousarray(
            xc.reshape(nt, P, 4, P).transpose(3, 2, 0, 1)
        ).reshape(P, 4 * nt * P).astype(BF)
        in_maps.append({"x": xT, "w": w, "b": b})

    from concourse.bass_utils import run_bass_kernel_spmd
    _ensure_axon_hooks()
    res = run_bass_kernel_spmd(nc, in_maps, core_ids=list(range(NCORES)))
    global last_result
    last_result = res

    s = plan["s"]
    idx_c = s % NCORES
    idx_p = s // NCORES
    # [8, nt*P, J*H] device outputs (bf16 h/c per layer)
    slabs = np.stack([np.asarray(res.results[c]["hc"]) for c in range(NCORES)])
    full = {}
    for jo, nm in enumerate(OUT_NAMES):
        full[nm] = slabs[idx_c, idx_p, jo * H:(jo + 1) * H].astype(np.float32)

    return (full["h4"], full["h1"], full["c1"], full["h2"], full["c2"],
            full["h3"], full["c3"], full["h4"], full["c4"])


if __name__ == "__main__":
    import reference
    inputs = reference.setup_inputs()
    out = kernel(**{k: np.asarray(v) for k, v in inputs.items()})
    print([o.shape for o in out])
